# revision 9
# baseline (speedup 1.0000x reference)
import numpy as np

NUM_TAGS = 32
START = 30
STOP = 31
B = 1024
S = 512
NCORES = 8
BC = 128          # batch per core
NG = 4            # partition groups
BW = 32           # batch per group (free dim width)
KQ = 6            # Q renorm period
KG = 16           # G renorm period
CH = 32           # timesteps per setup chunk

_BUILT = None


def _np_dt(mybir):
    return {
        "f32": mybir.dt.float32,
        "i32": mybir.dt.int32,
        "u8": mybir.dt.uint8,
        "bf16": mybir.dt.bfloat16,
    }


def build_module(S_=S, oht2_f32=True, debug_dump=False):
    import concourse.bass as bass
    import concourse.mybir as mybir
    from concourse import tile, bacc
    from concourse.alu_op_type import AluOpType
    from contextlib import ExitStack

    dt = _np_dt(mybir)
    F32, I32, U8 = dt["f32"], dt["i32"], dt["u8"]
    OHDT = F32 if oht2_f32 else dt["bf16"]
    AF = mybir.ActivationFunctionType
    NEP = (S_ - 1) // KQ + 1
    NCH = (S_ + CH - 1) // CH
    assert S_ % CH == 0 or S_ < CH

    nc = bacc.Bacc("TRN2", target_bir_lowering=False, debug=False, num_devices=NCORES)

    # ---- dram io ----
    feats_d = nc.dram_tensor("feats", [BC, S_ * 32], F32, kind="ExternalInput")
    labels_d = nc.dram_tensor("labels", [BC, S_], I32, kind="ExternalInput")
    lenvec_d = nc.dram_tensor("lenvec", [BC, 1], F32, kind="ExternalInput")
    ebd_d = nc.dram_tensor("ebd", [128, 128], F32, kind="ExternalInput")
    egbd_d = nc.dram_tensor("egbd", [128, 128], F32, kind="ExternalInput")
    onesbd_d = nc.dram_tensor("onesbd", [128, 4], F32, kind="ExternalInput")
    delta4_d = nc.dram_tensor("delta4", [4, 128], F32, kind="ExternalInput")
    expts_d = nc.dram_tensor("expts", [128, 1], F32, kind="ExternalInput")
    jrow_d = nc.dram_tensor("jrow", [128, 32], F32, kind="ExternalInput")
    startoh_d = nc.dram_tensor("startoh", [128, 32], F32, kind="ExternalInput")
    stopoh_d = nc.dram_tensor("stopoh", [128, 32], F32, kind="ExternalInput")
    iotat_d = nc.dram_tensor("iotat", [128, S_], F32, kind="ExternalInput")
    lenm1t_d = nc.dram_tensor("lenm1t", [128, 32], F32, kind="ExternalInput")
    lenept_d = nc.dram_tensor("lenept", [128, 32], F32, kind="ExternalInput")
    iotaep_d = nc.dram_tensor("iotaep", [128, NEP], F32, kind="ExternalInput")
    c31_d = nc.dram_tensor("c31", [128, 1], F32, kind="ExternalInput")
    c32_d = nc.dram_tensor("c32", [128, 1], F32, kind="ExternalInput")

    o_fsnum = nc.dram_tensor("o_fsnum", [4, 32], F32, kind="ExternalOutput")
    o_cf = nc.dram_tensor("o_cf", [128, 32], F32, kind="ExternalOutput")
    o_goldnum = nc.dram_tensor("o_goldnum", [4, 32], F32, kind="ExternalOutput")
    o_cg = nc.dram_tensor("o_cg", [128, 32], F32, kind="ExternalOutput")
    o_emit = nc.dram_tensor("o_emit", [BC, 1], F32, kind="ExternalOutput")
    NEP_ = (S_ - 1) // KQ + 1
    if debug_dump:
        o_dbg_emep = nc.dram_tensor("o_dbg_emep", [128, NEP_ * 32], U8, kind="ExternalOutput")
        o_dbg = nc.dram_tensor("o_dbg", [128, 6 * 32], F32, kind="ExternalOutput")

    with tile.TileContext(nc) as tc, ExitStack() as ctx:
        cpool = ctx.enter_context(tc.tile_pool(name="const", bufs=1))
        bpool = ctx.enter_context(tc.tile_pool(name="big", bufs=1))
        spool = ctx.enter_context(tc.tile_pool(name="state", bufs=1))
        chpool = ctx.enter_context(tc.tile_pool(name="chunk", bufs=3))
        scpool = ctx.enter_context(tc.tile_pool(name="scratch", bufs=2))
        rlpool = ctx.enter_context(tc.tile_pool(name="rl", bufs=2))
        psum = ctx.enter_context(
            tc.tile_pool(name="psum", bufs=2, space="PSUM")
        )
        psum4 = ctx.enter_context(
            tc.tile_pool(name="psum4", bufs=2, space="PSUM")
        )

        def cload(dram, shape, dtype, tag):
            t = cpool.tile(shape, dtype, tag=tag)
            nc.sync.dma_start(t[:], dram[:, :])
            return t

        ebd = cload(ebd_d, [128, 128], F32, "ebd")
        egbd = cload(egbd_d, [128, 128], F32, "egbd")
        onesbd = cload(onesbd_d, [128, 4], F32, "onesbd")
        delta4 = cload(delta4_d, [4, 128], F32, "delta4")
        expts = cload(expts_d, [128, 1], F32, "expts")
        jrow = cload(jrow_d, [128, 32], F32, "jrow")
        startoh = cload(startoh_d, [128, 32], F32, "startoh")
        stopoh = cload(stopoh_d, [128, 32], F32, "stopoh")
        iotat = cload(iotat_d, [128, S_], F32, "iotat")
        lenm1t = cload(lenm1t_d, [128, 32], F32, "lenm1t")
        lenept = cload(lenept_d, [128, 32], F32, "lenept")
        iotaep = cload(iotaep_d, [128, NEP], F32, "iotaep")
        c31 = cload(c31_d, [128, 1], F32, "c31")
        c32 = cload(c32_d, [128, 1], F32, "c32")
        labels = cload(labels_d, [BC, S_], I32, "labels")
        lenvec = cload(lenvec_d, [BC, 1], F32, "lenvec")

        # ---- masks / padded labels (f32 domain for compare ops) ----
        minv = cpool.tile([128, S_], U8, tag="minv")
        nc.vector.tensor_scalar(
            minv[:], iotat[:], lenvec[:], None, AluOpType.is_ge
        )
        lm32 = cpool.tile([128, S_], F32, tag="lm32")
        nc.gpsimd.tensor_copy(lm32[:], labels[:])
        nc.vector.copy_predicated(
            lm32[:], minv[:], c32[:, 0:1].broadcast_to([128, S_])
        )
        lm2 = cpool.tile([128, S_], F32, tag="lm2")
        nc.gpsimd.tensor_copy(lm2[:], labels[:])
        nc.vector.copy_predicated(
            lm2[:], minv[:], c31[:, 0:1].broadcast_to([128, S_])
        )

        # ---- EM: (len-1 == t) in transposed layout [(g,i), t*32+b] ----
        em = bpool.tile([128, S_ * 32], U8, tag="em")
        nc.vector.tensor_tensor(
            em[:].rearrange("p (t b) -> p t b", b=32),
            iotat[:].unsqueeze(2).broadcast_to([128, S_, 32]),
            lenm1t[:].unsqueeze(1).broadcast_to([128, S_, 32]),
            AluOpType.is_equal,
        )
        emep = cpool.tile([128, NEP * 32], U8, tag="emep")
        nc.vector.tensor_tensor(
            emep[:].rearrange("p (e b) -> p e b", b=32),
            iotaep[:].unsqueeze(2).broadcast_to([128, NEP, 32]),
            lenept[:].unsqueeze(1).broadcast_to([128, NEP, 32]),
            AluOpType.is_equal,
        )

        # ---- big residents ----
        fbig = bpool.tile([128, S_ * 32], F32, tag="fbig")
        oht2 = bpool.tile([128, S_ * 32], OHDT, tag="oht2")

        emit_parts = spool.tile([128, NCH], F32, tag="emit_parts")

        chw = min(CH, S_)
        for k in range(NCH):
            t0 = k * chw
            cols = chw * 32
            fc = chpool.tile([128, cols], F32, tag="fc")
            nc.sync.dma_start(fc[:], feats_d[:, t0 * 32 : t0 * 32 + cols])
            ftc = scpool.tile([128, cols], F32, tag="ftc")
            nc.vector.transpose(ftc[:], fc[:])
            nc.scalar.activation(
                fbig[:, t0 * 32 : t0 * 32 + cols], ftc[:], AF.Exp
            )
            # emit one-hot (valid labels only)
            ohc = scpool.tile([128, cols], F32, tag="ohc")
            nc.vector.tensor_tensor(
                ohc[:].rearrange("p (t j) -> p t j", j=32),
                lm32[:, t0 : t0 + chw].unsqueeze(2).broadcast_to([128, chw, 32]),
                jrow[:].unsqueeze(1).broadcast_to([128, chw, 32]),
                AluOpType.is_equal,
            )
            esc = scpool.tile([128, cols], F32, tag="esc")
            nc.vector.scalar_tensor_tensor(
                esc[:],
                ohc[:],
                1.0,
                fc[:],
                AluOpType.mult,
                AluOpType.mult,
                accum_out=emit_parts[:, k : k + 1],
            )
            # pair/ghost one-hot (invalid -> STOP), transposed
            oh2c = scpool.tile([128, cols], OHDT, tag="oh2c")
            nc.vector.tensor_tensor(
                oh2c[:].rearrange("p (t j) -> p t j", j=32),
                lm2[:, t0 : t0 + chw].unsqueeze(2).broadcast_to([128, chw, 32]),
                jrow[:].unsqueeze(1).broadcast_to([128, chw, 32]),
                AluOpType.is_equal,
            )
            nc.vector.transpose(oht2[:, t0 * 32 : t0 * 32 + cols], oh2c[:])

        emit_out = spool.tile([128, 1], F32, tag="emit_out")
        nc.vector.reduce_sum(
            emit_out[:], emit_parts[:], axis=mybir.AxisListType.X
        )
        nc.sync.dma_start(o_emit[:, :], emit_out[:])

        # ---- states ----
        q = spool.tile([128, 32], F32, tag="q")
        qf = spool.tile([128, 32], F32, tag="qf")
        cf = spool.tile([128, 32], F32, tag="cf")
        c128 = spool.tile([128, 32], F32, tag="c128")
        gst = spool.tile([128, 32], F32, tag="gst")
        cg128 = spool.tile([128, 32], F32, tag="cg128")

        dbg = None
        if debug_dump:
            dbg = spool.tile([128, 6 * 32], F32, tag="dbg")
            nc.vector.memset(dbg[:], 0.0)
        nc.vector.memset(qf[:], 0.0)
        nc.vector.memset(cf[:], 0.0)
        nc.vector.memset(c128[:], 0.0)
        nc.vector.memset(cg128[:], 0.0)

        # t = 0 init
        nc.vector.tensor_scalar_mul(q[:], fbig[:, 0:32], expts[:])
        nc.vector.copy_predicated(qf[:], em[:, 0:32], q[:])
        sg0 = psum.tile([128, 32], F32, tag="sg")
        nc.tensor.matmul(sg0[:], egbd[:], startoh[:])
        nc.vector.tensor_tensor(gst[:], sg0[:], oht2[:, 0:32], AluOpType.mult)

        def renorm(state, cstate, do_cf_ep=None):
            cs = psum4.tile([4, 32], F32, tag="cs")
            nc.tensor.matmul(cs[:], onesbd[:], state[:])
            rl = rlpool.tile([4, 64], F32, tag="rl")
            nc.vector.reciprocal(rl[:, 0:32], cs[:])
            nc.scalar.activation(rl[:, 32:64], cs[:], AF.Ln)
            rb = psum.tile([128, 64], F32, tag="rb")
            nc.tensor.matmul(rb[:], delta4[:], rl[:])
            if do_cf_ep is not None:
                e0 = do_cf_ep * 32
                nc.vector.copy_predicated(cf[:], emep[:, e0 : e0 + 32], c128[:])
            nc.vector.tensor_tensor(state[:], state[:], rb[:, 0:32], AluOpType.mult)
            nc.vector.tensor_tensor(
                cstate[:], cstate[:], rb[:, 32:64], AluOpType.add
            )

        for t in range(1, S_):
            c0 = t * 32
            sq = psum.tile([128, 32], F32, tag="sq")
            nc.tensor.matmul(sq[:], ebd[:], q[:])
            nc.vector.tensor_tensor(
                q[:], sq[:], fbig[:, c0 : c0 + 32], AluOpType.mult
            )
            nc.vector.copy_predicated(qf[:], em[:, c0 : c0 + 32], q[:])
            sg = psum.tile([128, 32], F32, tag="sg")
            nc.tensor.matmul(sg[:], egbd[:], gst[:])
            nc.vector.tensor_tensor(
                gst[:], sg[:], oht2[:, c0 : c0 + 32], AluOpType.mult
            )
            if t % KQ == KQ - 1 and t != S_ - 1:
                renorm(q, c128, do_cf_ep=t // KQ)
                if debug_dump and t // KQ < 2:
                    i0 = (t // KQ) * 2
                    nc.vector.tensor_copy(dbg[:, i0 * 32 : i0 * 32 + 32], cf[:])
                    nc.vector.tensor_copy(dbg[:, (i0 + 1) * 32 : (i0 + 1) * 32 + 32], c128[:])
            if t % KG == KG - 1 and t != S_ - 1:
                renorm(gst, cg128)

        # final CF capture for the last (un-renormalized) epoch(s)
        last_ren_ep = ((S_ - 2) // KQ) if S_ >= 2 else -1
        for ep in range(max(0, last_ren_ep), NEP):
            e0 = ep * 32
            nc.vector.copy_predicated(cf[:], emep[:, e0 : e0 + 32], c128[:])

        # G pad step t = S (STOP absorbing; adds stop transition for len==S)
        sgp = psum.tile([128, 32], F32, tag="sg")
        nc.tensor.matmul(sgp[:], egbd[:], gst[:])
        nc.vector.tensor_tensor(gst[:], sgp[:], stopoh[:], AluOpType.mult)

        # ---- outputs ----
        csqf = psum4.tile([4, 32], F32, tag="cs")
        nc.tensor.matmul(csqf[:], onesbd[:], qf[:])
        o1 = rlpool.tile([4, 32], F32, tag="o1")
        nc.scalar.copy(o1[:], csqf[:])
        nc.sync.dma_start(o_fsnum[:, :], o1[:])

        csg = psum4.tile([4, 32], F32, tag="cs")
        nc.tensor.matmul(csg[:], onesbd[:], gst[:])
        o3 = rlpool.tile([4, 32], F32, tag="o3")
        nc.scalar.copy(o3[:], csg[:])
        nc.sync.dma_start(o_goldnum[:, :], o3[:])

        if debug_dump:
            nc.vector.tensor_copy(dbg[:, 4 * 32 : 4 * 32 + 32], cf[:])
            nc.vector.tensor_copy(dbg[:, 5 * 32 : 5 * 32 + 32], c128[:])
            nc.sync.dma_start(o_dbg[:, :], dbg[:])
            nc.sync.dma_start(o_dbg_emep[:, :], emep[:])
        nc.sync.dma_start(o_cf[:, :], cf[:])
        nc.sync.dma_start(o_cg[:, :], cg128[:])

    nc.compile()
    return nc


def host_consts(transitions, lengths_c, S_=S):
    """Per-core small input tensors. lengths_c: [128] int32."""
    NEP = (S_ - 1) // KQ + 1
    tr = np.asarray(transitions, dtype=np.float32)
    E = np.exp(tr)
    EG = E.copy()
    EG[STOP, STOP] = 1.0
    ebd = np.zeros((128, 128), np.float32)
    egbd = np.zeros((128, 128), np.float32)
    for g in range(NG):
        ebd[g * 32 : g * 32 + 32, g * 32 : g * 32 + 32] = E
        egbd[g * 32 : g * 32 + 32, g * 32 : g * 32 + 32] = EG
    onesbd = np.zeros((128, 4), np.float32)
    for g in range(NG):
        onesbd[g * 32 : g * 32 + 32, g] = 1.0
    delta4 = np.zeros((4, 128), np.float32)
    for g in range(NG):
        delta4[g, g * 32 : g * 32 + 32] = 1.0
    expts = np.tile(np.exp(tr[START])[:, None], (NG, 1)).astype(np.float32)
    jrow = np.tile(np.arange(32, dtype=np.float32)[None, :], (128, 1))
    startoh = np.zeros((128, 32), np.float32)
    startoh[np.arange(128) % 32 == START] = 1.0
    stopoh = np.zeros((128, 32), np.float32)
    stopoh[np.arange(128) % 32 == STOP] = 1.0
    iotat = np.tile(np.arange(S_, dtype=np.float32)[None, :], (128, 1))
    iotaep = np.tile(np.arange(NEP, dtype=np.float32)[None, :], (128, 1))
    L = np.asarray(lengths_c, dtype=np.int32).reshape(NG, BW)
    lenm1t = np.tile((L - 1)[:, None, :], (1, 32, 1)).reshape(128, 32).astype(np.float32)
    lenept = np.tile(((L - 1) // KQ)[:, None, :], (1, 32, 1)).reshape(128, 32).astype(np.float32)
    c31 = np.full((128, 1), 31, np.float32)
    c32 = np.full((128, 1), 32, np.float32)
    return dict(
        ebd=ebd, egbd=egbd, onesbd=onesbd, delta4=delta4, expts=expts,
        jrow=jrow, startoh=startoh, stopoh=stopoh, iotat=iotat,
        lenm1t=lenm1t, lenept=lenept, iotaep=iotaep, c31=c31, c32=c32,
    )


def combine_outputs(res_c):
    """res_c: dict of output arrays for one core -> (fs_sum, gold_sum)."""
    fs = np.log(res_c["o_fsnum"].astype(np.float64)) + res_c["o_cf"][::32]
    gold_struct = np.log(res_c["o_goldnum"].astype(np.float64)) + res_c["o_cg"][::32]
    emit = res_c["o_emit"].astype(np.float64).reshape(NG, BW)
    return float(np.sum(fs)), float(np.sum(gold_struct) + np.sum(emit))


def kernel(feats, labels, lengths, transitions):
    global _BUILT
    feats = np.ascontiguousarray(np.asarray(feats, dtype=np.float32))
    labels = np.ascontiguousarray(np.asarray(labels, dtype=np.int32))
    lengths = np.ascontiguousarray(np.asarray(lengths, dtype=np.int32))
    transitions = np.asarray(transitions, dtype=np.float32)

    if _BUILT is None:
        _BUILT = build_module(S)
    nc = _BUILT

    from concourse.bass_utils import run_bass_kernel_spmd

    in_maps = []
    for c in range(NCORES):
        sl = slice(c * BC, (c + 1) * BC)
        m = dict(
            feats=feats[sl].reshape(BC, S * 32),
            labels=labels[sl],
            lenvec=lengths[sl].reshape(BC, 1).astype(np.float32),
        )
        m.update(host_consts(transitions, lengths[sl], S))
        in_maps.append(m)

    res = run_bass_kernel_spmd(nc, in_maps, list(range(NCORES)))

    tot_fs = 0.0
    tot_gold = 0.0
    for c in range(NCORES):
        f, g = combine_outputs(res.results[c])
        tot_fs += f
        tot_gold += g
    return np.float32((tot_fs - tot_gold) / B)
